# revision 1
# baseline (speedup 1.0000x reference)
"""Attention-distillation KL loss on 8 Trainium2 NeuronCores.

Math: the reference softmaxes + L2-normalizes every row of student_out
[500000, 128], but the scalar loss only reads the rows gathered by
node_ids [256] and neighbor_idx [256, 32].  softmax and l2-normalize are
per-row, so they commute with the gather; furthermore
    sf = softmax(x) / ||softmax(x)|| = exp(x) / ||exp(x)||
(the softmax denominator and any max-shift cancel in the L2 norm), and
exp never overflows for N(0,1) logits.  So each core only has to:

  - exp the raw gathered rows,
  - compute cosine sims  sim[m,k] = <e_node[m], e_nbr[m,k]> / (||e_node[m]|| ||e_nbr[m,k]||),
  - masked log-softmax over k for student sims and teacher weights,
  - per-node KL.

Sharding: 256 sampled nodes -> 32 per core.  Per core the 32*32 = 1024
(m, k) pairs are laid out pair-major on SBUF partitions: 8 column bands
of [128 partitions x 128 classes]; pair q = m*32+k lives in band q//128,
partition q%128.  The node row is replicated across its 32 k-partitions
(host-side np.repeat), which makes every step a plain elementwise /
free-dim-reduce op - no transposes, no partition broadcasts.

Per-node reductions over k (32 partitions in a group) use one PE matmul
with a [128, 4] group-indicator matrix:  Z = G^T @ [ems | emt | w]
where ems = mask*exp(sim), emt = mask*exp(teacher), w = emt*(teacher-sim).
With Zs/Zt the masked-softmax denominators and U the w-sums, per-node
KL is  kl[m] = U/Zt + log(Zs/Zt)  (uses sum_k t_dist = 1).  Each core
returns its [4, 24] Z tile; the host finishes the 32 log/div per core
and reduces the scalar loss in float64.
"""

import numpy as np
from contextlib import ExitStack

import concourse.bass as bass
import concourse.tile as tile
from concourse import bacc, mybir
from concourse.bass_utils import run_bass_kernel_spmd

N_CORES = 8
M, K, C = 256, 32, 128
MPC = M // N_CORES            # nodes per core
PAIRS = MPC * K               # 1024 (m,k) pairs per core
T = PAIRS // 128              # 8 column bands
FREE = T * C                  # 1024 free-dim elements per partition
NGRP = 128 // K               # 4 nodes per band

# column offsets inside the packed small input "sm"
SM_TW = 0                     # [128, T]   teacher pairs
SM_MK = SM_TW + T             # [128, T]   mask pairs
SM_G = SM_MK + T              # [128, 4]   G[p, g] = (p//32 == g)
SM_XN = SM_G + NGRP           # [32, C]    raw node rows (parts 0:32)
SM_GT = SM_XN + C             # [4, 128]   G^T (parts 0:4)
SM_G4 = SM_GT + 128           # [32, 4]    G4[m, g] = (m%4 == g)
SM_I8 = SM_G4 + NGRP          # [32, 8]    ind8[m, t] = (m//4 == t)
SM_W = SM_I8 + T

_cache = {}


def _patch_act_tables():
    """Make Exp/Ln/Square resolve only to the combined
    natural_log_exp_and_others table set, so the whole kernel needs a
    single ACT_TABLE_LOAD instead of thrashing exp<->ln sets (~1.3us per
    switch)."""
    if _cache.get("act_patched"):
        return
    orig = bacc.get_activation_tables
    combined = "natural_log_exp_and_others"
    special = {mybir.ActivationFunctionType.Exp,
               mybir.ActivationFunctionType.Ln,
               mybir.ActivationFunctionType.Square}

    def patched(arch):
        tabs = orig(arch)
        if combined in tabs and special <= tabs[combined]:
            for name, fns in tabs.items():
                if name != combined:
                    fns -= special
        return tabs

    bacc.get_activation_tables = patched
    _cache["act_patched"] = True


def _build_nc():
    _patch_act_tables()
    nc = bacc.Bacc("TRN2", target_bir_lowering=False, debug=False,
                   enable_asserts=True, num_devices=N_CORES)
    f32 = mybir.dt.float32
    Exp = mybir.ActivationFunctionType.Exp
    Log = mybir.ActivationFunctionType.Ln

    xa = nc.dram_tensor("xa", [128, FREE], f32, kind="ExternalInput").ap()
    xb = nc.dram_tensor("xb", [128, FREE], f32, kind="ExternalInput").ap()
    # sm packs [tw | mk | G | xn | GT | G4 | ind8] -> one small DMA
    sm = nc.dram_tensor("sm", [128, SM_W], f32, kind="ExternalInput").ap()
    zo = nc.dram_tensor("zo", [NGRP, 3 * T], f32, kind="ExternalOutput").ap()

    with tile.TileContext(nc) as tc, ExitStack() as ctx:
        sb = ctx.enter_context(tc.tile_pool(name="sb", bufs=1))
        ps = ctx.enter_context(tc.tile_pool(name="ps", bufs=1, space="PSUM"))

        H = FREE // 2
        TH = T // 2
        h0 = slice(0, H)
        h1 = slice(H, FREE)
        Square = mybir.ActivationFunctionType.Square

        # xa/xb ride the Sync HWDGE as four 256KB halves (first halves
        # interleaved so compute starts while the rest is in flight);
        # the small packed sm rides the GpSimd SWDGE in parallel.
        sxa = [sb.tile([128, H], f32, name=f"sxa{h}") for h in range(2)]
        sxb = [sb.tile([128, H], f32, name=f"sxb{h}") for h in range(2)]
        ssm = sb.tile([128, SM_W], f32)
        nc.gpsimd.dma_start(ssm[:], sm[:, :])
        for h in range(2):
            nc.sync.dma_start(sxa[h][:], xa[:, h * H:(h + 1) * H])
            nc.sync.dma_start(sxb[h][:], xb[:, h * H:(h + 1) * H])
        stw, smk = ssm[:, SM_TW:SM_TW + T], ssm[:, SM_MK:SM_MK + T]
        sg = ssm[:, SM_G:SM_G + NGRP]

        ea = sb.tile([128, FREE], f32)
        eb = sb.tile([128, FREE], f32)
        prod_a = sb.tile([128, FREE], f32)
        prod_q = sb.tile([128, FREE], f32)
        red_a = sb.tile([128, T], f32)
        red_q = sb.tile([128, T], f32)
        cat = sb.tile([128, 3 * T], f32)

        # ScalarE static queue: ea0, eb0 first (so GpSimd's products
        # start earliest), tiny teacher/node exps woven in, then the
        # second halves.  VectorE keeps only the reduces and the sim
        # chain; GpSimd takes the cross-products, squares-h0 and the
        # small muls; ScalarE does the node square+sum via accum_out.
        nc.scalar.activation(ea[:, h0], sxa[0][:], Exp)
        et = sb.tile([128, T], f32)
        en = sb.tile([MPC, C], f32)
        nc.scalar.activation(et[:], stw[:], Exp)
        nc.scalar.activation(en[:], ssm[0:MPC, SM_XN:SM_XN + C], Exp)
        nc.scalar.activation(eb[:, h0], sxb[0][:], Exp)
        nc.scalar.activation(ea[:, h1], sxa[1][:], Exp)
        nc.scalar.activation(prod_a[:, h1], ea[:, h1], Square)
        nc.scalar.activation(eb[:, h1], sxb[1][:], Exp)
        en2 = sb.tile([MPC, C], f32)
        n2b = sb.tile([MPC, 1], f32)
        nc.scalar.activation(en2[:], en[:], Square, accum_out=n2b[:])
        lnb = sb.tile([MPC, 1], f32)
        nc.scalar.activation(lnb[:], n2b[:], Log)
        rqb = sb.tile([MPC, 1], f32)
        nc.scalar.activation(rqb[:], lnb[:], Exp, scale=-0.5)

        # GpSimd: big products + the small elementwise muls
        emt = cat[:, T:2 * T]
        nc.gpsimd.tensor_mul(prod_a[:, h0], ea[:, h0], ea[:, h0])
        nc.gpsimd.tensor_mul(prod_q[:, h0], ea[:, h0], eb[:, h0])
        nc.gpsimd.tensor_mul(emt, et[:], smk[:])
        nc.gpsimd.tensor_mul(prod_q[:, h1], ea[:, h1], eb[:, h1])
        vg = sb.tile([MPC, NGRP], f32)
        nc.gpsimd.tensor_scalar_mul(vg[:], ssm[0:MPC, SM_G4:SM_G4 + NGRP],
                                    rqb[:])

        # node-side inverse-norm broadcast to pair layout [128, T] with
        # two tiny matmuls:
        #   z1[g, t]  = sum_m (rqb[m]*G4[m, g]) * ind8[m, t] = rqb[4t+g]
        #   rqbp[p,t] = sum_g GT[g, p] * z1[g, t]            = rqb[node(p,t)]
        z1 = ps.tile([NGRP, T], f32)
        nc.tensor.matmul(z1[:], vg[:], ssm[0:MPC, SM_I8:SM_I8 + T])
        z1s = sb.tile([NGRP, T], f32)
        nc.scalar.copy(z1s[:], z1[:])
        rqbp = ps.tile([128, T], f32)
        nc.tensor.matmul(rqbp[:], ssm[0:NGRP, SM_GT:SM_GT + 128], z1s[:])

        def _red(dst, src, h):
            nc.vector.reduce_sum(
                dst[:, h * TH:(h + 1) * TH],
                src[:, h * H:(h + 1) * H].rearrange("p (t c) -> p t c", c=C),
                axis=mybir.AxisListType.X,
            )

        _red(red_a, prod_a, 0)
        _red(red_q, prod_q, 0)
        _red(red_a, prod_a, 1)
        _red(red_q, prod_q, 1)
        n2a, raw = red_a[:, 0:T], red_q[:, 0:T]

        # rqa = 1/sqrt(n2a) via exp(-0.5*ln); Exp/Ln are ~2 ULP.
        lg = sb.tile([128, T], f32)
        nc.scalar.activation(lg[:], n2a, Log)
        rqa = sb.tile([128, T], f32)
        nc.scalar.activation(rqa[:], lg[:], Exp, scale=-0.5)

        s1 = sb.tile([128, T], f32)
        nc.vector.tensor_mul(s1[:], raw, rqbp[:])
        sim = sb.tile([128, T], f32)
        nc.vector.tensor_mul(sim[:], s1[:], rqa[:])

        # cat = [mask*exp(sim) | emt | emt*(tw - sim)]
        es = sb.tile([128, T], f32)
        nc.scalar.activation(es[:], sim[:], Exp)
        nc.gpsimd.tensor_mul(cat[:, 0:T], es[:], smk[:])
        dd = sb.tile([128, T], f32)
        nc.gpsimd.tensor_sub(dd[:], stw[:], sim[:])
        nc.vector.tensor_mul(cat[:, 2 * T:3 * T], emt, dd[:])

        # group-of-32-partitions sums:  [Zs | Zt | U] = G^T @ cat.
        # The final 32 values/core of kl[m] = U/Zt + log(Zs/Zt) are
        # finished on the host as part of the loss reduction.
        z = ps.tile([NGRP, 3 * T], f32)
        nc.tensor.matmul(z[:], sg[:], cat[:])
        zc = sb.tile([NGRP, 3 * T], f32)
        nc.scalar.copy(zc[:], z[:])
        nc.sync.dma_start(zo[:, :], zc[:])

    nc.compile()
    return nc


def _get_nc():
    if "nc" not in _cache:
        _cache["nc"] = _build_nc()
    return _cache["nc"]


def _band_layout(a):
    """[PAIRS, C] row-major -> [128, T*C] band layout (band t cols hold
    pair rows 128t..128t+127)."""
    return np.ascontiguousarray(
        a.reshape(T, 128, C).transpose(1, 0, 2).reshape(128, FREE))


def _cols_layout(a):
    """[PAIRS] -> [128, T] with column t = pairs 128t..128t+127."""
    return np.ascontiguousarray(a.reshape(T, 128).T)


def _make_in_maps(student_out, teacher_weights, node_ids, neighbor_idx,
                  neighbor_mask):
    student_out = np.asarray(student_out, dtype=np.float32)
    teacher_weights = np.asarray(teacher_weights, dtype=np.float32)
    node_ids = np.asarray(node_ids).astype(np.int64)
    neighbor_idx = np.asarray(neighbor_idx).astype(np.int64)
    mask_f = np.asarray(neighbor_mask).astype(np.float32)

    gg = np.zeros((128, NGRP), dtype=np.float32)
    gg[np.arange(128), np.arange(128) // K] = 1.0

    in_maps = []
    for c in range(N_CORES):
        ms = slice(MPC * c, MPC * (c + 1))
        a_rows = student_out[neighbor_idx[ms].reshape(-1)]        # [1024, C]
        b_rows = np.repeat(student_out[node_ids[ms]], K, axis=0)  # [1024, C]
        sm = np.zeros((128, SM_W), dtype=np.float32)
        sm[:, SM_TW:SM_TW + T] = _cols_layout(teacher_weights[ms].reshape(-1))
        sm[:, SM_MK:SM_MK + T] = _cols_layout(mask_f[ms].reshape(-1))
        sm[:, SM_G:SM_G + NGRP] = gg
        sm[0:MPC, SM_XN:SM_XN + C] = student_out[node_ids[ms]]
        sm[0:NGRP, SM_GT:SM_GT + 128] = gg.T
        sm[0:MPC, SM_G4:SM_G4 + NGRP] = (
            np.arange(MPC)[:, None] % NGRP == np.arange(NGRP)[None, :])
        sm[0:MPC, SM_I8:SM_I8 + T] = (
            np.arange(MPC)[:, None] // NGRP == np.arange(T)[None, :])
        in_maps.append({
            "xa": _band_layout(a_rows),
            "xb": _band_layout(b_rows),
            "sm": sm,
        })
    return in_maps


def _run(in_maps, **kwargs):
    try:
        return run_bass_kernel_spmd(_get_nc(), in_maps,
                                    core_ids=list(range(N_CORES)), **kwargs)
    except Exception:
        # one retry for transient device hiccups
        return run_bass_kernel_spmd(_get_nc(), in_maps,
                                    core_ids=list(range(N_CORES)), **kwargs)


def _per_node_kl(results):
    """results -> per-node kl [M] in node order (float64 host finish)."""
    kl = np.empty(M, dtype=np.float64)
    for c in range(N_CORES):
        z = results[c]["zo"].astype(np.float64)   # [NGRP, 3T]; node = 4t+g
        zs, zt, u = z[:, 0:T], z[:, T:2 * T], z[:, 2 * T:3 * T]
        knode = u / zt + np.log(zs / zt)          # [NGRP, T]
        kl[MPC * c: MPC * (c + 1)] = knode.T.reshape(-1)
    return kl


def kernel(student_out, teacher_weights, node_ids, neighbor_idx,
           neighbor_mask):
    in_maps = _make_in_maps(student_out, teacher_weights, node_ids,
                            neighbor_idx, neighbor_mask)
    res = _run(in_maps)
    kl = _per_node_kl(res.results)
    return np.asarray(kl.sum() / M, dtype=np.float32)



# revision 2
# speedup vs baseline: 1.4484x; 1.4484x over previous
"""Attention-distillation KL loss on 8 Trainium2 NeuronCores.

Math: the reference softmaxes + L2-normalizes every row of student_out
[500000, 128], but the scalar loss only reads the rows gathered by
node_ids [256] and neighbor_idx [256, 32].  softmax and l2-normalize are
per-row, so they commute with the gather; furthermore
    sf = softmax(x) / ||softmax(x)|| = exp(x) / ||exp(x)||
(the softmax denominator and any max-shift cancel in the L2 norm).  So
per (node m, neighbor k) pair with raw rows xb=x[node], xa=x[nbr]:

    sim[m,k] = sum_c exp(xa+xb) / (||exp(xa)|| * ||exp(xb)||)

The node-side norm is per-node (only 256 rows), so the host folds it
additively into a combined logit tensor
    xs[q, c] = xa[q, c] + xn[m(q), c] - 0.5*ln(sum_c exp(2*xn[m(q)]))
and the device computes, per 128-partition band layout (pair q = 128t+p
on partition p, band t; q = 32*m + k node-major):

    rawb = segreduce_c exp(xs)            -> sim numerator * rqb   [128,8]
    n2a' = segreduce_c exp(2*xa - S)      -> nbr sq-norm * e^-S    [128,8]
    rqa  = exp(-0.5*(ln n2a' + S))        -> 1/||exp(xa)||
    sim  = rawb * rqa
    ems  = exp(sim)*mask ; w = emt*(tw - sim)   (emt = exp(tw)*mask, host)

The shift S=4 keeps exp(2*xa-S) inside fp16 range.  The device ships
cat = [ems | emt | w] [128, 24]; the host finishes the tiny [256, 32]
per-node masked-softmax sums and KL in float64 (Zs=sum_k ems etc.,
kl = U/Zt + log(Zs/Zt), using sum_k t_dist = 1), as the baseline did.

Engine budget per core: 2 big fp16 exps on ScalarE, 2 1x segment
reductions on VectorE, ~6 tiny [128,8] ops, 4 fp16 in-DMAs (512KB) on
the Sync HWDGE ring + 2 small ones on GpSimd SWDGE, one 6KB out-DMA.
No PE, no PSUM.
"""

import numpy as np
from contextlib import ExitStack

import concourse.bass as bass
import concourse.tile as tile
from concourse import bacc, mybir
from concourse.bass_utils import run_bass_kernel_spmd

N_CORES = 8
M, K, C = 256, 32, 128
MPC = M // N_CORES            # nodes per core
PAIRS = MPC * K               # 1024 (m,k) pairs per core
T = PAIRS // 128              # 8 column bands
FREE = T * C                  # 1024 free-dim elements per partition
H = FREE // 2
TH = T // 2
SHIFT = 4.0                   # exp(2*xa - SHIFT) fp16 overflow guard

# smA (f32) column map: [tw | mk | zero | -S/2]
SA_TW = 0
SA_MK = SA_TW + T
SA_Z = SA_MK + T              # 0.0 bias column
SA_HS = SA_Z + 1              # -SHIFT/2 bias column
SA_W = SA_HS + 1

_cache = {}


def _patch_act_tables():
    """Make Exp/Ln resolve only to the combined natural_log_exp_and_others
    table set, so the whole kernel needs a single ACT_TABLE_LOAD instead of
    thrashing exp<->ln sets (~2.7us per switch)."""
    if _cache.get("act_patched"):
        return
    orig = bacc.get_activation_tables
    combined = "natural_log_exp_and_others"
    special = {mybir.ActivationFunctionType.Exp,
               mybir.ActivationFunctionType.Ln,
               mybir.ActivationFunctionType.Square}

    def patched(arch):
        tabs = orig(arch)
        if combined in tabs and special <= tabs[combined]:
            for name, fns in tabs.items():
                if name != combined:
                    fns -= special
        return tabs

    bacc.get_activation_tables = patched
    _cache["act_patched"] = True


def _build_nc():
    _patch_act_tables()
    nc = bacc.Bacc("TRN2", target_bir_lowering=False, debug=False,
                   enable_asserts=True, num_devices=N_CORES)
    f32 = mybir.dt.float32
    f16 = mybir.dt.float16
    Exp = mybir.ActivationFunctionType.Exp
    Ln = mybir.ActivationFunctionType.Ln

    xa = nc.dram_tensor("xa", [128, FREE], f16, kind="ExternalInput").ap()
    xs = nc.dram_tensor("xs", [128, FREE], f16, kind="ExternalInput").ap()
    sma = nc.dram_tensor("sma", [128, SA_W], f32, kind="ExternalInput").ap()
    smb = nc.dram_tensor("smb", [128, 3 * T], f16, kind="ExternalInput").ap()
    zo = nc.dram_tensor("zo", [128, 3 * T], f16, kind="ExternalOutput").ap()

    with tile.TileContext(nc) as tc, ExitStack() as ctx:
        sb = ctx.enter_context(tc.tile_pool(name="sb", bufs=1))

        sxa = sb.tile([128, FREE], f16)
        sxs = sb.tile([128, FREE], f16)
        sa = sb.tile([128, SA_W], f32)
        cat = sb.tile([128, 3 * T], f16)

        # All big in-DMAs ride the Sync HWDGE ring (ACT stays free for
        # compute); the two small tensors ride the GpSimd SWDGE.
        h0 = slice(0, H)
        h1 = slice(H, FREE)
        nc.sync.dma_start(sxa[:, h0], xa[:, h0])
        nc.sync.dma_start(sxs[:, h0], xs[:, h0])
        nc.sync.dma_start(sxa[:, h1], xa[:, h1])
        nc.sync.dma_start(sxs[:, h1], xs[:, h1])
        nc.gpsimd.dma_start(sa[:], sma[:, :])
        nc.gpsimd.dma_start(cat[:], smb[:, :])

        zc = sa[:, SA_Z:SA_Z + 1]
        hsc = sa[:, SA_HS:SA_HS + 1]
        stw = sa[:, SA_TW:SA_TW + T]
        smk = sa[:, SA_MK:SA_MK + T]

        sq = sb.tile([128, FREE], f16)
        es = sb.tile([128, FREE], f16)
        n2a = sb.tile([128, T], f32)
        rawb = sb.tile([128, T], f32)

        # ScalarE: 4 half-tensor exps, woven so each starts as soon as its
        # DMA half lands; VectorE reduces trail each exp.
        nc.scalar.activation(sq[:, h0], sxa[:, h0], Exp, scale=2.0, bias=zc)
        nc.scalar.activation(es[:, h0], sxs[:, h0], Exp, bias=zc)
        nc.scalar.activation(sq[:, h1], sxa[:, h1], Exp, scale=2.0, bias=zc)
        nc.scalar.activation(es[:, h1], sxs[:, h1], Exp, bias=zc)

        def _red(dst, src, h):
            nc.vector.reduce_sum(
                dst[:, h * TH:(h + 1) * TH],
                src[:, h * H:(h + 1) * H].rearrange("p (t c) -> p t c", c=C),
                axis=mybir.AxisListType.X,
            )

        _red(n2a, sq, 0)
        _red(n2a, sq, 1)
        _red(rawb, es, 0)
        _red(rawb, es, 1)

        # rqa = 1/sqrt(n2a * e^SHIFT) = exp(-0.5*(ln n2a' + SHIFT))
        lg = sb.tile([128, T], f32)
        nc.scalar.activation(lg[:], n2a[:], Ln, bias=zc)
        rqa = sb.tile([128, T], f32)
        nc.scalar.activation(rqa[:], lg[:], Exp, scale=-0.5, bias=hsc)

        sim = sb.tile([128, T], f32)
        nc.vector.tensor_mul(sim[:], rawb[:], rqa[:])
        es2 = sb.tile([128, T], f32)
        nc.scalar.activation(es2[:], sim[:], Exp, bias=zc)

        # cat = [ems | emt (host) | w]
        nc.vector.tensor_mul(cat[:, 0:T], es2[:], smk)
        dd = sb.tile([128, T], f32)
        nc.vector.tensor_sub(dd[:], stw, sim[:])
        nc.vector.tensor_mul(cat[:, 2 * T:3 * T], cat[:, T:2 * T], dd[:])

        nc.sync.dma_start(zo[:, :], cat[:])

    nc.compile()
    return nc


def _get_nc():
    if "nc" not in _cache:
        _cache["nc"] = _build_nc()
    return _cache["nc"]


def _band_layout(a):
    """[PAIRS, C] row-major -> [128, T*C] band layout (band t cols hold
    pair rows 128t..128t+127)."""
    return np.ascontiguousarray(
        a.reshape(T, 128, C).transpose(1, 0, 2).reshape(128, FREE))


def _cols_layout(a):
    """[PAIRS] -> [128, T] with column t = pairs 128t..128t+127."""
    return np.ascontiguousarray(a.reshape(T, 128).T)


def _make_in_maps(student_out, teacher_weights, node_ids, neighbor_idx,
                  neighbor_mask):
    student_out = np.asarray(student_out, dtype=np.float32)
    teacher_weights = np.asarray(teacher_weights, dtype=np.float32)
    node_ids = np.asarray(node_ids).astype(np.int64)
    neighbor_idx = np.asarray(neighbor_idx).astype(np.int64)
    mask_f = np.asarray(neighbor_mask).astype(np.float32)

    in_maps = []
    emt_all = []
    for c in range(N_CORES):
        ms = slice(MPC * c, MPC * (c + 1))
        a_rows = student_out[neighbor_idx[ms].reshape(-1)]        # [1024, C]
        xn = student_out[node_ids[ms]].astype(np.float64)         # [32, C]
        lnb = -0.5 * np.log(np.exp(2.0 * xn).sum(axis=1))         # [32]
        xbp = (xn + lnb[:, None]).astype(np.float32)              # [32, C]
        xs_rows = a_rows + np.repeat(xbp, K, axis=0)              # [1024, C]

        tw = teacher_weights[ms].reshape(-1)                      # [1024]
        mk = mask_f[ms].reshape(-1)
        emt = np.exp(teacher_weights[ms].astype(np.float64)) * mask_f[ms]
        emt_all.append(emt)                                       # [32, 32]

        sma = np.zeros((128, SA_W), dtype=np.float32)
        sma[:, SA_TW:SA_TW + T] = _cols_layout(tw)
        sma[:, SA_MK:SA_MK + T] = _cols_layout(mk)
        sma[:, SA_HS] = -SHIFT / 2.0
        smb = np.zeros((128, 3 * T), dtype=np.float16)
        smb[:, T:2 * T] = _cols_layout(emt.reshape(-1)).astype(np.float16)

        in_maps.append({
            "xa": _band_layout(a_rows).astype(np.float16),
            "xs": _band_layout(xs_rows).astype(np.float16),
            "sma": sma,
            "smb": smb,
        })
    _cache["emt_all"] = emt_all
    return in_maps


def _run(in_maps, **kwargs):
    try:
        return run_bass_kernel_spmd(_get_nc(), in_maps,
                                    core_ids=list(range(N_CORES)), **kwargs)
    except Exception:
        # one retry for transient device hiccups
        return run_bass_kernel_spmd(_get_nc(), in_maps,
                                    core_ids=list(range(N_CORES)), **kwargs)


def _per_node_kl(results):
    """results -> per-node kl [M] in node order (float64 host finish)."""
    kl = np.empty(M, dtype=np.float64)
    for c in range(N_CORES):
        z = results[c]["zo"].astype(np.float64)   # [128, 3T] band layout
        # column t holds pairs 128t..128t+127 (q = 32m + k node-major)
        ems = z[:, 0:T].T.reshape(MPC, K)
        w = z[:, 2 * T:3 * T].T.reshape(MPC, K)
        emt = _cache["emt_all"][c]                # exact f64 host copy
        zs = ems.sum(axis=1)
        zt = emt.sum(axis=1)
        u = w.sum(axis=1)
        kl[MPC * c: MPC * (c + 1)] = u / zt + np.log(zs / zt)
    return kl


def kernel(student_out, teacher_weights, node_ids, neighbor_idx,
           neighbor_mask):
    in_maps = _make_in_maps(student_out, teacher_weights, node_ids,
                            neighbor_idx, neighbor_mask)
    res = _run(in_maps)
    kl = _per_node_kl(res.results)
    return np.asarray(kl.sum() / M, dtype=np.float32)
